# revision 19
# baseline (speedup 1.0000x reference)
"""LEConvMultiEdge Trainium2 kernel (8 NeuronCores, SPMD data-parallel).

Math (per batch b, dest node i, channel c):
  out = sigmoid(V@w1 + sum_l deg_l * (V@w2_l) - sum_l A_l @ (V@w3_l))
  deg_l[i] = sum_j A[b,i,j,l]

Device strategy: shard the 4096 (b,i) destination rows over 8 cores (512
each). The host pre-arranges each core's A shard as At[J-tile, l, j, i] in
fp8 e3m4 so the contraction dim (j) lands on SBUF partitions and each
128-row j-tile chunk feeds one accumulating matmul. The stationary operand
for chunk (J,l) is [U3'_l(J) | e_l] (68 wide) where U3' = V@(-w3) so the
chain accumulates -term3 directly and the one-hot e_l columns produce the
per-edge-type degree rows in the same PSUM bank. U3' is built on-device as
16 independent tiles (one matmul + one cast each) so the chain starts as
soon as tile 0 and the first A group have landed. term1 (V@w1) accumulates
into the same bank via one fp32r matmul; term2's V@w2_l run as two fp32r
matmuls overlapped with the chain, then deg rows are broadcast across
partitions with two tiny outer-product matmuls and combined on DVE. The
output is produced transposed [C, i]; the host transposes back for free.
"""

import sys

if "/opt/trn_rl_repo" not in sys.path:
    sys.path.insert(0, "/opt/trn_rl_repo")

import numpy as np

B, N, F, C, L = 2, 2048, 64, 64, 4
P = 128
NCORES = 8
SH_PER_B = NCORES // B  # 4 shards per batch entry
IPC = N // SH_PER_B  # 512 dest rows per core
NJT = N // P  # 16 j-tiles
NCHUNK = L * NJT  # 64 contraction chunks
SW = C + L  # stationary width: 64 U3 cols + 4 deg one-hot cols
LSW = L * SW  # 272

# A stream dtype. fp8 e3m4 (1-3-4) keeps ~1.2 decimal digits; measured
# end-to-end rel err 5.7e-3 vs the 2e-2 gate (bf16: 8.3e-4 at 2x the HBM
# traffic). "bf16" is the fallback mode.
MODE = "fp8"

_NC_CACHE = {}


def _mode_params(mode):
    # group = one 1 MiB DMA of chunks; chunk = [128 j, 512 i]
    if mode == "fp8":
        grp = 16
    else:
        grp = 8
    ngrp = NCHUNK // grp
    jpg = grp // L  # j-tiles consumed per group (J-outer chunk order)
    return grp, ngrp, jpg


def _build_nc(mode=MODE):
    import concourse.bacc as bacc
    import concourse.bass as bass
    import concourse.mybir as mybir
    import concourse.tile as tile

    dt = mybir.dt.float32
    dtr = mybir.dt.float32r
    dtb = mybir.dt.bfloat16
    dta = mybir.dt.float8e3 if mode == "fp8" else mybir.dt.bfloat16
    GRP, NGRP, JPG = _mode_params(mode)

    nc = bacc.Bacc("TRN2", debug=False, target_bir_lowering=False, num_devices=NCORES)

    At = nc.dram_tensor("At", [NGRP, P, GRP * IPC], dta, kind="ExternalInput")
    # pka: [65, 2048 | 272] bf16 = [V^T ; ones] | [-w3 packed ; one-hot]
    PKA = nc.dram_tensor("PKA", [F + 1, N + LSW], dtb, kind="ExternalInput")
    # pkb: [64, 512 | 64 | 256 | 256] f32r = V[i-shard]^T | w1 | w2 packed |
    # deg-broadcast selector (rows 0:4)
    PKB = nc.dram_tensor("PKB", [F, IPC + C + L * C + 2 * P], dtr, kind="ExternalInput")
    # fold: cols 0:64 sum the two 64-row blocks (fold[p, m] = (p%64 == m));
    # cols 64:128 identity on rows 0:64 (injects term1-term3 into the same
    # PSUM group)
    FOLD = nc.dram_tensor("FOLD", [P, 2 * C], dtr, kind="ExternalInput")
    out_d = nc.dram_tensor("out", [C, IPC], dt, kind="ExternalOutput")

    VTO, W1O, W2O, SELO = 0, IPC, IPC + C, IPC + C + L * C

    with tile.TileContext(nc) as tc:
        with (
            tc.tile_pool(name="const", bufs=1) as constp,
            tc.tile_pool(name="ats", bufs=3) as atp,
            tc.tile_pool(name="pacc", bufs=1, space=bass.MemorySpace.PSUM) as pacc,
            tc.tile_pool(name="pu2", bufs=1, space=bass.MemorySpace.PSUM) as pu2,
            tc.tile_pool(name="pbc", bufs=1, space=bass.MemorySpace.PSUM) as pbc,
            tc.tile_pool(name="ps", bufs=1, space=bass.MemorySpace.PSUM) as ps,
            tc.tile_pool(name="psub", bufs=2, space=bass.MemorySpace.PSUM) as psub,
            tc.tile_pool(name="work", bufs=1) as work,
        ):
            # ---- packed const loads (2 DMAs)
            pka = constp.tile([F + 1, N + LSW], dtb)
            nc.sync.dma_start(pka[:], PKA[:])
            pkb = constp.tile([F, IPC + C + L * C + 2 * P], dtr)
            nc.sync.dma_start(pkb[:], PKB[:])
            foldc = constp.tile([P, 2 * C], dtr)
            nc.sync.dma_start(foldc[:], FOLD[:])

            # ---- A stream DMAs (1 MiB each, triple-buffered)
            ats = []
            for g in range(NGRP):
                at = atp.tile([P, GRP * IPC], dta)
                nc.sync.dma_start(at[:], At[g])
                ats.append(at)

            # ---- U3' build: 16 independent tiles, one matmul + one cast
            # each. ub = [vte_J]^T @ w3pe -> [128 j, 272]; the appended ones
            # row of vte times the one-hot row of w3pe plants exact 1.0s in
            # the deg columns.
            ujs = []
            for J in range(NJT):
                ub = psub.tile([P, LSW], dt, tag="ub")
                nc.tensor.matmul(
                    ub[:],
                    pka[:, J * P : (J + 1) * P],
                    pka[:, N : N + LSW],
                    start=True,
                    stop=True,
                )
                uj = constp.tile([P, LSW], dta, tag=f"uj{J}")
                nc.vector.tensor_copy(uj[:], ub[:])
                ujs.append(uj)

            # ---- big contraction + fused term1 / u2 matmuls
            acc = pacc.tile([SW, IPC], dt)
            ua = pu2.tile([P, IPC], dt, tag="ua")
            ub2 = pu2.tile([P, IPC], dt, tag="ub2")
            s1 = ps.tile([C, IPC], dt, tag="s1")
            q = 0
            for g in range(NGRP):
                at = ats[g]
                for c4 in range(GRP):
                    J = g * JPG + c4 // L
                    l = c4 % L
                    nc.tensor.matmul(
                        acc[:],
                        ujs[J][:, l * SW : (l + 1) * SW],
                        at[:, c4 * IPC : (c4 + 1) * IPC],
                        start=(q == 0),
                        stop=(q == NCHUNK - 1),
                    )
                    q += 1
                if g == 0:
                    # term1 opens the s1 accumulation group (the fold matmuls
                    # close it in the epilogue); u2 halves go to their own
                    # banks. All fp32r (1 cyc/row at 512 moving cols),
                    # slotted while group 1 streams.
                    nc.tensor.matmul(
                        s1[:],
                        pkb[:, W1O : W1O + C],
                        pkb[:, VTO : VTO + IPC],
                        start=True,
                        stop=False,
                    )
                    nc.tensor.matmul(
                        ua[:],
                        pkb[:, W2O : W2O + P],
                        pkb[:, VTO : VTO + IPC],
                        start=True,
                        stop=True,
                    )
                    nc.tensor.matmul(
                        ub2[:],
                        pkb[:, W2O + P : W2O + 2 * P],
                        pkb[:, VTO : VTO + IPC],
                        start=True,
                        stop=True,
                    )
                    # park u2 in SBUF mid-chain (DVE idle here); the epilogue
                    # muls may read only one PSUM operand
                    uas = work.tile([P, IPC], dt, tag="uas")
                    ub2s = work.tile([P, IPC], dt, tag="ub2s")
                    nc.vector.tensor_copy(uas[:], ua[:])
                    nc.vector.tensor_copy(ub2s[:], ub2[:])

            # ---- epilogue, all in [*, i] orientation, base-0 partitions only
            degs = work.tile([L, IPC], dtr, tag="degs")
            nc.vector.tensor_copy(degs[:], acc[C:SW, :])
            sacc = work.tile([C, IPC], dtr, tag="sacc")
            nc.vector.tensor_copy(sacc[:], acc[0:C, :])
            # broadcast deg rows across partitions: bca rows = deg0|deg1,
            # bcb rows = deg2|deg3
            bca = pbc.tile([P, IPC], dt, tag="bca")
            bcb = pbc.tile([P, IPC], dt, tag="bcb")
            nc.tensor.matmul(
                bca[:], pkb[0:L, SELO : SELO + P], degs[:], start=True, stop=True
            )
            nc.tensor.matmul(
                bcb[:], pkb[0:L, SELO + P : SELO + 2 * P], degs[:], start=True, stop=True
            )
            tmpa = work.tile([P, IPC], dtr, tag="tmpa")
            tmpb = work.tile([P, IPC], dtr, tag="tmpb")
            nc.vector.tensor_mul(tmpa[:], uas[:], bca[:])
            nc.vector.tensor_mul(tmpb[:], ub2s[:], bcb[:])
            # s1 += term2 (block-folded deg*u2) + (-term3 from sacc);
            # term1 already opened this group mid-chain
            nc.tensor.matmul(s1[:], foldc[:, 0:C], tmpa[:], start=False, stop=False)
            nc.tensor.matmul(s1[:], foldc[:, 0:C], tmpb[:], start=False, stop=False)
            nc.tensor.matmul(
                s1[:], foldc[0:C, C : 2 * C], sacc[:], start=False, stop=True
            )
            o = work.tile([C, IPC], dt, tag="o")
            nc.scalar.activation(o[:], s1[:], mybir.ActivationFunctionType.Sigmoid)
            nc.sync.dma_start(out_d[:], o[:])

    nc.compile()
    return nc


def _get_nc(mode=None):
    if mode is None:
        mode = MODE
    key = ("nc", mode)
    if key not in _NC_CACHE:
        _NC_CACHE[key] = _build_nc(mode)
    return _NC_CACHE[key]


def _shard_inputs(V, A, w1, w2, w3, mode=None):
    import ml_dtypes

    if mode is None:
        mode = MODE
    GRP, NGRP, JPG = _mode_params(mode)
    dta_np = ml_dtypes.float8_e3m4 if mode == "fp8" else ml_dtypes.bfloat16
    bf16 = ml_dtypes.bfloat16

    V = np.ascontiguousarray(np.asarray(V, dtype=np.float32))
    A = np.asarray(A, dtype=np.float32)
    w1 = np.asarray(w1, dtype=np.float32)
    w2 = np.asarray(w2, dtype=np.float32)
    w3 = np.asarray(w3, dtype=np.float32)

    # w3pe [65, 272]: per l block, cols 0:64 = -w3_l, col 64+l = one-hot row
    w3pe = np.zeros((F + 1, LSW), dtype=np.float32)
    for l in range(L):
        w3pe[0:F, l * SW : l * SW + C] = -w3[l * F : (l + 1) * F, :]
        w3pe[F, l * SW + C + l] = 1.0
    # w2 packed (l f) c -> f (l c)
    w2p = np.zeros((F, L * C), dtype=np.float32)
    for l in range(L):
        w2p[:, l * C : (l + 1) * C] = w2[l * F : (l + 1) * F, :]
    # deg-broadcast selector [64, 256]: rows 0:4 hold the one-hot pattern,
    # cols 0:128 select deg rows (0,1), cols 128:256 rows (2,3)
    selp = np.zeros((F, 2 * P), dtype=np.float32)
    selp[0, 0:C] = 1.0
    selp[1, C : 2 * C] = 1.0
    selp[2, P : P + C] = 1.0
    selp[3, P + C : P + 2 * C] = 1.0
    # fold [128, 128]: cols 0:64 block-sum, cols 64:128 identity on rows 0:64
    foldp = np.zeros((P, 2 * C), dtype=np.float32)
    for p in range(P):
        foldp[p, p % C] = 1.0
    for p in range(C):
        foldp[p, C + p] = 1.0

    in_maps = []
    for k in range(NCORES):
        b, sshard = divmod(k, SH_PER_B)
        i0 = sshard * IPC
        # A chunk order: group g holds J = g*JPG + (0..JPG-1), l-fast
        Asl = A[b, i0 : i0 + IPC]  # (IPC, N, L) = (i, j, l)
        At3 = Asl.transpose(2, 1, 0)  # (l, j, i)
        t = At3.reshape(L, NGRP, JPG, P, IPC)
        Atg = t.transpose(1, 3, 2, 0, 4).reshape(NGRP, P, GRP * IPC)
        # pka: [V^T ; ones] | w3pe, bf16
        vte = np.concatenate(
            [V[b].T, np.ones((1, N), dtype=np.float32)], axis=0
        )  # (65, N)
        pka = np.concatenate([vte, w3pe], axis=1)  # (65, N + 272)
        # pkb: vto | w1 | w2p, f32
        vto = V[b, i0 : i0 + IPC].T  # (64, 512)
        pkb = np.concatenate([vto, w1, w2p, selp], axis=1)  # (64, 1088)
        in_maps.append(
            {
                "At": np.ascontiguousarray(Atg).astype(dta_np),
                "PKA": np.ascontiguousarray(pka.astype(bf16)),
                "PKB": np.ascontiguousarray(pkb),
                "FOLD": foldp,
            }
        )
    return in_maps


LAST_EXEC_NS = None


def kernel(V, A, w1, w2, w3, _trace=False):
    global LAST_EXEC_NS
    from concourse.bass_utils import run_bass_kernel_spmd

    nc = _get_nc()
    in_maps = _shard_inputs(V, A, w1, w2, w3)
    res = run_bass_kernel_spmd(nc, in_maps, list(range(NCORES)), trace=_trace)
    LAST_EXEC_NS = res.exec_time_ns
    out = np.empty((B, N, C), dtype=np.float32)
    for k in range(NCORES):
        b, sshard = divmod(k, SH_PER_B)
        i0 = sshard * IPC
        out[b, i0 : i0 + IPC] = res.results[k]["out"].T
    return out


# revision 30
# speedup vs baseline: 1.0207x; 1.0207x over previous
"""LEConvMultiEdge Trainium2 kernel (8 NeuronCores, SPMD data-parallel).

Math (per batch b, dest node i, channel c):
  out = sigmoid(V@w1 + sum_l deg_l * (V@w2_l) - sum_l A_l @ (V@w3_l))
  deg_l[i] = sum_j A[b,i,j,l]

Device strategy: shard the 4096 (b,i) destination rows over 8 cores (512
each). The host pre-arranges each core's A shard as At[J-tile, l, j, i] in
fp8 e3m4 so the contraction dim (j) lands on SBUF partitions and each
128-row j-tile chunk feeds one accumulating matmul. The stationary operand
for chunk (J,l) is [U3'_l(J) | e_l] (68 wide) where U3' = V@(-w3) so the
chain accumulates -term3 directly and the one-hot e_l columns produce the
per-edge-type degree rows in the same PSUM bank. U3' is built on-device as
16 independent tiles (one matmul + one cast each) so the chain starts as
soon as tile 0 and the first A group have landed. term1 (V@w1) accumulates
into the same bank via one fp32r matmul; term2's V@w2_l run as two fp32r
matmuls overlapped with the chain, then deg rows are broadcast across
partitions with two tiny outer-product matmuls and combined on DVE. The
output is produced transposed [C, i]; the host transposes back for free.
"""

import sys

if "/opt/trn_rl_repo" not in sys.path:
    sys.path.insert(0, "/opt/trn_rl_repo")

import numpy as np

B, N, F, C, L = 2, 2048, 64, 64, 4
P = 128
NCORES = 8
SH_PER_B = NCORES // B  # 4 shards per batch entry
IPC = N // SH_PER_B  # 512 dest rows per core
NJT = N // P  # 16 j-tiles
NCHUNK = L * NJT  # 64 contraction chunks
SW = C + L  # stationary width: 64 U3 cols + 4 deg one-hot cols
LSW = L * SW  # 272

# A stream dtype. fp8 e3m4 (1-3-4) keeps ~1.2 decimal digits; measured
# end-to-end rel err 5.7e-3 vs the 2e-2 gate (bf16: 8.3e-4 at 2x the HBM
# traffic). "bf16" is the fallback mode.
MODE = "fp8"

_NC_CACHE = {}


# A-stream DMA group sizes in chunks (chunk = [128 j, 512 i]). Small leading
# groups let the chain start as soon as uj_0 is built; big trailing groups
# keep DMA efficiency. Chunks are ordered J-outer, l-fast.
GROUPS = [2, 2, 4, 8, 16, 16, 16]
assert sum(GROUPS) == NCHUNK


def _build_nc(mode=MODE):
    import concourse.bacc as bacc
    import concourse.bass as bass
    import concourse.mybir as mybir
    import concourse.tile as tile

    dt = mybir.dt.float32
    dtr = mybir.dt.float32r
    dtb = mybir.dt.bfloat16
    dta = mybir.dt.float8e3 if mode == "fp8" else mybir.dt.bfloat16

    nc = bacc.Bacc("TRN2", debug=False, target_bir_lowering=False, num_devices=NCORES)

    At = nc.dram_tensor("At", [P, NCHUNK * IPC], dta, kind="ExternalInput")
    # pka: [65, 2048 | 272] bf16 = [V^T ; ones] | [-w3 packed ; one-hot]
    PKA = nc.dram_tensor("PKA", [F + 1, N + LSW], dtb, kind="ExternalInput")
    # pkb: [64, 512 | 64 | 256 | 256] f32r = V[i-shard]^T | w1 | w2 packed |
    # deg-broadcast selector (rows 0:4)
    PKB = nc.dram_tensor("PKB", [F, IPC + C + L * C + 2 * P], dtr, kind="ExternalInput")
    # fold: cols 0:64 sum the two 64-row blocks (fold[p, m] = (p%64 == m));
    # cols 64:128 identity on rows 0:64 (injects term1-term3 into the same
    # PSUM group)
    FOLD = nc.dram_tensor("FOLD", [P, 2 * C], dtr, kind="ExternalInput")
    out_d = nc.dram_tensor("out", [C, IPC], dt, kind="ExternalOutput")

    VTO, W1O, W2O, SELO = 0, IPC, IPC + C, IPC + C + L * C

    with tile.TileContext(nc) as tc:
        with (
            tc.tile_pool(name="const", bufs=1) as constp,
            tc.tile_pool(name="ats", bufs=1) as atp,
            tc.tile_pool(name="pacc", bufs=1, space=bass.MemorySpace.PSUM) as pacc,
            tc.tile_pool(name="pu2", bufs=1, space=bass.MemorySpace.PSUM) as pu2,
            tc.tile_pool(name="pbc", bufs=1, space=bass.MemorySpace.PSUM) as pbc,
            tc.tile_pool(name="ps", bufs=1, space=bass.MemorySpace.PSUM) as ps,
            tc.tile_pool(name="psub", bufs=2, space=bass.MemorySpace.PSUM) as psub,
            tc.tile_pool(name="work", bufs=1) as work,
        ):
            # ---- packed const loads (2 DMAs)
            pka = constp.tile([F + 1, N + LSW], dtb)
            nc.sync.dma_start(pka[:], PKA[:])
            pkb = constp.tile([F, IPC + C + L * C + 2 * P], dtr)
            nc.sync.dma_start(pkb[:], PKB[:])
            foldc = constp.tile([P, 2 * C], dtr)
            nc.sync.dma_start(foldc[:], FOLD[:])

            # ---- A stream DMAs. Early groups ride the second HWDGE ring
            # (scalar) in parallel with the const loads on the sync ring;
            # late groups go behind the consts on sync so the scalar queue
            # frees up for the odd uj casts. The whole shard fits in SBUF
            # (32 KB/partition fp8), so no buffer reuse / WAR stalls.
            ats = []
            off = 0
            for gi, gsz in enumerate(GROUPS):
                at = atp.tile([P, gsz * IPC], dta, tag=f"at{gi}")
                eng = nc.scalar if gi < 3 else nc.sync
                eng.dma_start(at[:], At[:, off * IPC : (off + gsz) * IPC])
                ats.append(at)
                off += gsz

            # ---- U3' build: 16 independent tiles, one matmul + one cast
            # each. ub = [vte_J]^T @ w3pe -> [128 j, 272]; the appended ones
            # row of vte times the one-hot row of w3pe plants exact 1.0s in
            # the deg columns.
            ujs = []
            for J in range(NJT):
                ub = psub.tile([P, LSW], dt, tag="ub")
                nc.tensor.matmul(
                    ub[:],
                    pka[:, J * P : (J + 1) * P],
                    pka[:, N : N + LSW],
                    start=True,
                    stop=True,
                )
                uj = constp.tile([P, LSW], dta, tag=f"uj{J}")
                # alternate cast engines so the build never gates the chain
                if J % 2 == 0:
                    nc.vector.tensor_copy(uj[:], ub[:])
                else:
                    nc.scalar.activation(
                        uj[:], ub[:], mybir.ActivationFunctionType.Copy
                    )
                ujs.append(uj)

            # ---- big contraction + fused term1 / u2 matmuls
            acc = pacc.tile([SW, IPC], dt)
            ua = pu2.tile([P, IPC], dt, tag="ua")
            ub2 = pu2.tile([P, IPC], dt, tag="ub2")
            s1 = ps.tile([C, IPC], dt, tag="s1")
            q = 0
            for gi, gsz in enumerate(GROUPS):
                at = ats[gi]
                for c4 in range(gsz):
                    J, l = divmod(q, L)
                    nc.tensor.matmul(
                        acc[:],
                        ujs[J][:, l * SW : (l + 1) * SW],
                        at[:, c4 * IPC : (c4 + 1) * IPC],
                        start=(q == 0),
                        stop=(q == NCHUNK - 1),
                    )
                    q += 1
                if gi == 2:
                    # term1 opens the s1 accumulation group (the fold matmuls
                    # close it in the epilogue); u2 halves go to their own
                    # banks. All fp32r (1 cyc/row at 512 moving cols),
                    # slotted while group 1 streams.
                    nc.tensor.matmul(
                        s1[:],
                        pkb[:, W1O : W1O + C],
                        pkb[:, VTO : VTO + IPC],
                        start=True,
                        stop=False,
                    )
                    nc.tensor.matmul(
                        ua[:],
                        pkb[:, W2O : W2O + P],
                        pkb[:, VTO : VTO + IPC],
                        start=True,
                        stop=True,
                    )
                    nc.tensor.matmul(
                        ub2[:],
                        pkb[:, W2O + P : W2O + 2 * P],
                        pkb[:, VTO : VTO + IPC],
                        start=True,
                        stop=True,
                    )
                    # park u2 in SBUF mid-chain (DVE idle here); the epilogue
                    # muls may read only one PSUM operand
                    uas = work.tile([P, IPC], dt, tag="uas")
                    ub2s = work.tile([P, IPC], dt, tag="ub2s")
                    nc.vector.tensor_copy(uas[:], ua[:])
                    nc.vector.tensor_copy(ub2s[:], ub2[:])

            # ---- epilogue, all in [*, i] orientation, base-0 partitions only
            degs = work.tile([L, IPC], dtr, tag="degs")
            nc.vector.tensor_copy(degs[:], acc[C:SW, :])
            sacc = work.tile([C, IPC], dtr, tag="sacc")
            nc.vector.tensor_copy(sacc[:], acc[0:C, :])
            # broadcast deg rows across partitions: bca rows = deg0|deg1,
            # bcb rows = deg2|deg3
            bca = pbc.tile([P, IPC], dt, tag="bca")
            bcb = pbc.tile([P, IPC], dt, tag="bcb")
            nc.tensor.matmul(
                bca[:], pkb[0:L, SELO : SELO + P], degs[:], start=True, stop=True
            )
            nc.tensor.matmul(
                bcb[:], pkb[0:L, SELO + P : SELO + 2 * P], degs[:], start=True, stop=True
            )
            tmpa = work.tile([P, IPC], dtr, tag="tmpa")
            tmpb = work.tile([P, IPC], dtr, tag="tmpb")
            nc.vector.tensor_mul(tmpa[:], uas[:], bca[:])
            nc.vector.tensor_mul(tmpb[:], ub2s[:], bcb[:])
            # s1 += term2 (block-folded deg*u2) + (-term3 from sacc);
            # term1 already opened this group mid-chain
            nc.tensor.matmul(s1[:], foldc[:, 0:C], tmpa[:], start=False, stop=False)
            nc.tensor.matmul(s1[:], foldc[:, 0:C], tmpb[:], start=False, stop=False)
            nc.tensor.matmul(
                s1[:], foldc[0:C, C : 2 * C], sacc[:], start=False, stop=True
            )
            o = work.tile([C, IPC], dt, tag="o")
            nc.scalar.activation(o[:], s1[:], mybir.ActivationFunctionType.Sigmoid)
            nc.sync.dma_start(out_d[:], o[:])

    nc.compile()
    return nc


def _get_nc(mode=None):
    if mode is None:
        mode = MODE
    key = ("nc", mode)
    if key not in _NC_CACHE:
        _NC_CACHE[key] = _build_nc(mode)
    return _NC_CACHE[key]


def _shard_inputs(V, A, w1, w2, w3, mode=None):
    import ml_dtypes

    if mode is None:
        mode = MODE
    dta_np = ml_dtypes.float8_e3m4 if mode == "fp8" else ml_dtypes.bfloat16
    bf16 = ml_dtypes.bfloat16

    V = np.ascontiguousarray(np.asarray(V, dtype=np.float32))
    A = np.asarray(A, dtype=np.float32)
    w1 = np.asarray(w1, dtype=np.float32)
    w2 = np.asarray(w2, dtype=np.float32)
    w3 = np.asarray(w3, dtype=np.float32)

    # w3pe [65, 272]: per l block, cols 0:64 = -w3_l, col 64+l = one-hot row
    w3pe = np.zeros((F + 1, LSW), dtype=np.float32)
    for l in range(L):
        w3pe[0:F, l * SW : l * SW + C] = -w3[l * F : (l + 1) * F, :]
        w3pe[F, l * SW + C + l] = 1.0
    # w2 packed (l f) c -> f (l c)
    w2p = np.zeros((F, L * C), dtype=np.float32)
    for l in range(L):
        w2p[:, l * C : (l + 1) * C] = w2[l * F : (l + 1) * F, :]
    # deg-broadcast selector [64, 256]: rows 0:4 hold the one-hot pattern,
    # cols 0:128 select deg rows (0,1), cols 128:256 rows (2,3)
    selp = np.zeros((F, 2 * P), dtype=np.float32)
    selp[0, 0:C] = 1.0
    selp[1, C : 2 * C] = 1.0
    selp[2, P : P + C] = 1.0
    selp[3, P + C : P + 2 * C] = 1.0
    # fold [128, 128]: cols 0:64 block-sum, cols 64:128 identity on rows 0:64
    foldp = np.zeros((P, 2 * C), dtype=np.float32)
    for p in range(P):
        foldp[p, p % C] = 1.0
    for p in range(C):
        foldp[p, C + p] = 1.0

    in_maps = []
    for k in range(NCORES):
        b, sshard = divmod(k, SH_PER_B)
        i0 = sshard * IPC
        # flat A layout [p, (J, l, i)]: chunk q = J*L + l, J-outer l-fast
        Asl = A[b, i0 : i0 + IPC]  # (IPC, N, L) = (i, j, l)
        At3 = Asl.transpose(2, 1, 0)  # (l, j, i)
        t = At3.reshape(L, NJT, P, IPC)
        Atg = t.transpose(2, 1, 0, 3).reshape(P, NCHUNK * IPC)
        # pka: [V^T ; ones] | w3pe, bf16
        vte = np.concatenate(
            [V[b].T, np.ones((1, N), dtype=np.float32)], axis=0
        )  # (65, N)
        pka = np.concatenate([vte, w3pe], axis=1)  # (65, N + 272)
        # pkb: vto | w1 | w2p, f32
        vto = V[b, i0 : i0 + IPC].T  # (64, 512)
        pkb = np.concatenate([vto, w1, w2p, selp], axis=1)  # (64, 1088)
        in_maps.append(
            {
                "At": np.ascontiguousarray(Atg).astype(dta_np),
                "PKA": np.ascontiguousarray(pka.astype(bf16)),
                "PKB": np.ascontiguousarray(pkb),
                "FOLD": foldp,
            }
        )
    return in_maps


LAST_EXEC_NS = None


def kernel(V, A, w1, w2, w3, _trace=False):
    global LAST_EXEC_NS
    from concourse.bass_utils import run_bass_kernel_spmd

    nc = _get_nc()
    in_maps = _shard_inputs(V, A, w1, w2, w3)
    res = run_bass_kernel_spmd(nc, in_maps, list(range(NCORES)), trace=_trace)
    LAST_EXEC_NS = res.exec_time_ns
    out = np.empty((B, N, C), dtype=np.float32)
    for k in range(NCORES):
        b, sshard = divmod(k, SH_PER_B)
        i0 = sshard * IPC
        out[b, i0 : i0 + IPC] = res.results[k]["out"].T
    return out
